# revision 17
# baseline (speedup 1.0000x reference)
"""Cayley orthogonal transform kernel for Trainium2 (8 NeuronCores).

Math: per head h, y = (I - S) ((1+eps) I + S)^{-1} x applied along D=128,
where S = S_raw - S_raw^T is skew-symmetric.  With A = (1+eps)I + S and
G = A^T = (1+eps)I - S, the applied matrix satisfies
    W = (I - S) A^{-1} = (2+eps) A^{-1} - I,
so only A^{-1} = G^T (G G^T)^{-1} is needed.  U = (G G^T)^{-1} is computed
with a Newton-Schulz iteration in residual form on the SPD matrix P = G G^T:
    E <- E^2,  U <- U + U E        (E0 = I - cP, U0 = cI)
where everything commutes (polynomials in P), letting both products be one
128x256 matmul per iteration (U|E packed side by side).  The two heads'
chains are interleaved instruction-by-instruction so they pipeline.

Schedule (per core = 2 heads, tensor parallel over heads):
  * Host: skew-symmetrize S_raw, lay x out as xT[h, d, token] bf16, and pack
    [S_h0 | S_h1 | I] into one f32 tensor (single small DMA on the ACT ring;
    HWDGE rings run DMAs serially, so it must stay off the x ring; the packed
    identity avoids an on-device iota).
  * Device: x half-panel DMAs (4 x 2 MiB) are issued up front on the Sync
    ring into distinct SBUF tiles (no reuse -> no WAR stalls), streaming from
    t~4us under the NS preamble.  Panels are 512-column bf16 matmuls into
    double-bank PSUM tiles; one 1024-column bf16 evacuation per tile
    alternates Vector/Scalar (amortizing the ~200ns per-op overhead); output
    DMAs are issued from GpSimd (SWDGE) so the ACT queue stays dedicated to
    evacuations; the final stores taper to shrink the drain tail.
  * Both x and y travel as single bf16 tensors (~3e-3 rel error, well within
    tolerance): ~16.8 MB per core at the per-core HBM roofline.
  * Host: upcast to fp32 + inverse layout transform back to (B, H, N, D).
"""

import os
import sys

import numpy as np

B, H, N, D = 4, 16, 4096, 128
N_CORES = 8
HPC = H // N_CORES          # heads per core
T = B * N                   # tokens per head
PIECE = 8192                # columns per input DMA (2 MiB bf16)
OUT_PIECE = 4096            # columns per output DMA (1 MiB bf16)
MM = 512                    # columns per matmul (one PSUM bank)
EV = 1024                   # columns per PSUM evacuation (two banks)
NS_ITERS = 4                # Newton-Schulz iterations (residual 0.6^16 ~ 3e-4)
NS_C = 0.4                  # NS scale: safe for ||S||_2 < ~2 (actual ~1.72)
EPS = 1e-5

_CACHE = {}


def _ensure_path():
    for p in ("/opt/trn_rl_repo", "/root/.axon_site/_ro/trn_rl_repo"):
        if os.path.isdir(p) and p not in sys.path:
            sys.path.insert(0, p)
    _install_ntff_hook()


def _install_ntff_hook():
    """The agent image's ``antenv`` lacks ``axon_hooks``, which makes
    ``run_bass_kernel_spmd(trace=True)`` crash instead of degrading.  Provide
    the module and register the ctypes NTFF hook the boot shim would have."""
    if "antenv.axon_hooks" in sys.modules:
        return
    try:
        import types

        import antenv

        if hasattr(antenv, "axon_hooks"):
            return
        mod = types.ModuleType("antenv.axon_hooks")
        state = {"hook": None}
        mod.set_axon_ntff_profile_hook = lambda h: state.__setitem__("hook", h)
        mod.get_axon_ntff_profile_hook = lambda: state["hook"]
        sys.modules["antenv.axon_hooks"] = mod
        antenv.axon_hooks = mod
        try:
            from trn_agent_boot.trn_boot import _ntff_profile_via_ctypes

            so_path = "/opt/axon/libaxon_pjrt.so"
            if os.path.exists(so_path):
                mod.set_axon_ntff_profile_hook(_ntff_profile_via_ctypes(so_path))
        except Exception:
            pass  # hook stays None -> concourse logs + skips tracing
    except Exception:
        pass


def _build_nc():
    """Build the (single-program SPMD) Bass kernel for one core's shard."""
    _ensure_path()
    import concourse.tile as tile
    from concourse import bacc, mybir

    f32 = mybir.dt.float32
    bf16 = mybir.dt.bfloat16
    Alu = mybir.AluOpType

    nc = bacc.Bacc("TRN2", target_bir_lowering=False, debug=False)
    x_d = nc.dram_tensor("x", [HPC * D, T], bf16, kind="ExternalInput").ap()
    # [S_h0 | S_h1 | I] packed: one DMA, no on-device identity construction
    s_d = nc.dram_tensor("s", [D, (HPC + 1) * D], f32,
                         kind="ExternalInput").ap()
    yT_d = nc.dram_tensor("yT", [HPC * D, T], bf16, kind="ExternalOutput").ap()

    n_pieces = T // PIECE

    with tile.TileContext(nc) as tc:
        with (
            tc.tile_pool(name="const", bufs=1) as const_pool,
            tc.tile_pool(name="ns", bufs=4) as ns_pool,
            tc.tile_pool(name="xin", bufs=1) as in_pool,
            tc.tile_pool(name="yout", bufs=1) as out_pool,
            tc.tile_pool(name="psns", bufs=2, space="PSUM") as ps_ns,
            tc.tile_pool(name="psmm", bufs=6, space="PSUM") as ps_mm,
        ):
            # s-pack first on the Sync ring: earliest-available DMA
            # path (~7us framework preamble on every ring; SWDGE is even
            # later).  It costs the h1 x-stream ~2us, which is off the
            # critical path.
            spack = const_pool.tile([D, (HPC + 1) * D], f32, tag="spack")
            nc.sync.dma_start(out=spack, in_=s_d)
            # every x half-panel up front, split across BOTH HWDGE
            # rings (head0 on ACT, head1 on Sync) so input streams at the
            # combined rate and finishes ~10us sooner
            xin = {}
            for h in range(HPC):
                for p in range(n_pieces):
                    t_ = in_pool.tile([D, PIECE], bf16, tag=f"x{h}_{p}",
                                      name=f"x{h}_{p}")
                    c0 = p * PIECE
                    eng = nc.scalar if h == 0 else nc.sync  # h0:ACT h1:SP
                    eng.dma_start(
                        out=t_, in_=x_d[h * D:(h + 1) * D, c0:c0 + PIECE])
                    xin[(h, p)] = t_

            ident = spack[:, HPC * D:(HPC + 1) * D]

            # ---- Newton-Schulz, heads interleaved per step
            a_mat, g_mat, ue = [], [], []
            for h in range(HPC):
                s_sl = spack[:, h * D:(h + 1) * D]
                a_ = const_pool.tile([D, D], f32, tag=f"amat{h}",
                                     name=f"amat{h}")
                nc.vector.scalar_tensor_tensor(
                    out=a_, in0=ident, scalar=1.0 + EPS, in1=s_sl,
                    op0=Alu.mult, op1=Alu.add)          # A = (1+eps)I + S
                a_mat.append(a_)
                g_ = const_pool.tile([D, D], f32, tag=f"gmat{h}",
                                     name=f"gmat{h}")
                nc.vector.scalar_tensor_tensor(
                    out=g_, in0=ident, scalar=1.0 + EPS, in1=s_sl,
                    op0=Alu.mult, op1=Alu.subtract)     # G = A^T
                g_mat.append(g_)

            ue0s = []
            for h in range(HPC):
                ue0 = ns_pool.tile([D, 2 * D], f32, tag=f"ue{h}",
                                   name=f"ue{h}")
                nc.vector.tensor_scalar_mul(ue0[:, 0:D], ident, NS_C)  # U0=cI
                ue0s.append(ue0)
            p0 = []
            for h in range(HPC):
                ps = ps_ns.tile([D, D], f32, tag="nsps", name="nsps")
                # lhsT=A -> A^T A = G G^T = P
                nc.tensor.matmul(ps, lhsT=a_mat[h], rhs=a_mat[h],
                                 start=True, stop=True)
                p0.append(ps)
            for h in range(HPC):
                ue0 = ue0s[h]
                nc.vector.scalar_tensor_tensor(
                    out=ue0[:, D:2 * D], in0=p0[h], scalar=-NS_C, in1=ident,
                    op0=Alu.mult, op1=Alu.add)          # E0 = I - cP
                ue.append(ue0)

            for k in range(NS_ITERS):
                prods = []
                for h in range(HPC):
                    ps = ps_ns.tile([D, 2 * D], f32, tag="nsps", name="nsps")
                    # lhsT = E (symmetric): [E U | E E] = [U E | E^2]
                    nc.tensor.matmul(ps, lhsT=ue[h][:, D:2 * D], rhs=ue[h],
                                     start=True, stop=True)
                    prods.append(ps)
                for h in range(HPC):
                    nxt = ns_pool.tile([D, 2 * D], f32, tag=f"ue{h}",
                                       name=f"ue{h}")
                    nc.vector.tensor_add(nxt[:, 0:D], ue[h][:, 0:D],
                                         prods[h][:, 0:D])      # U += U E
                    nc.scalar.copy(nxt[:, D:2 * D], prods[h][:, D:2 * D])
                    ue[h] = nxt

            wts = []
            for h in range(HPC):
                ps = ps_ns.tile([D, D], f32, tag="nsps", name="nsps")
                # lhsT=G -> G^T U = A U = A^{-1}
                nc.tensor.matmul(ps, lhsT=g_mat[h], rhs=ue[h][:, 0:D],
                                 start=True, stop=True)
                w_ = const_pool.tile([D, D], bf16, tag=f"w{h}", name=f"w{h}")
                nc.vector.scalar_tensor_tensor(
                    out=w_, in0=ps, scalar=2.0 + EPS, in1=ident,
                    op0=Alu.mult, op1=Alu.subtract)  # W^T = (2+eps)A^{-1} - I
                wts.append(w_)

            # ---- streaming panel matmuls: yT[h] = W @ xT[h] (all-bf16 I/O)
            ev_idx = 0
            for h in range(HPC):
                yo = {}
                for p in range(n_pieces):
                    yo[p] = out_pool.tile([D, PIECE], bf16, tag=f"y{h}_{p}",
                                          name=f"y{h}_{p}")
                for p in range(n_pieces):
                    xt, yt = xin[(h, p)], yo[p]
                    for j in range(PIECE // MM):
                        sl = slice(j * MM, (j + 1) * MM)
                        if ev_idx % 4 == 3:  # borrow the (idle) NS banks ->
                            ps = ps_ns.tile([D, MM], f32, tag="nsps",
                                            name="mmps")  # 8-deep rotation
                        else:
                            ps = ps_mm.tile([D, MM], f32, tag="mm",
                                            name="mmps")
                        nc.tensor.matmul(ps, lhsT=wts[h], rhs=xt[:, sl],
                                         start=True, stop=True)
                        if ev_idx % 2 == 0:
                            nc.vector.tensor_copy(yt[:, sl], ps)
                        else:
                            nc.scalar.copy(yt[:, sl], ps)
                        ev_idx += 1
                        col_end = (j + 1) * MM
                        r = slice(h * D, (h + 1) * D)
                        last_piece = (h == HPC - 1 and p == n_pieces - 1)
                        if last_piece and col_end % (2 * MM) == 0:
                            # fine-grained drain: 1024-col stores alternate
                            # between the sync ring and SWDGE so the final
                            # bytes drain on two paths in parallel
                            o0 = col_end - 2 * MM
                            g0 = p * PIECE + o0
                            oeng = nc.sync if (col_end // (2 * MM)) % 2 \
                                else nc.gpsimd
                            oeng.dma_start(
                                out=yT_d[r, g0:g0 + 2 * MM],
                                in_=yt[:, o0:o0 + 2 * MM])
                        elif not last_piece and col_end % OUT_PIECE == 0:
                            o0 = col_end - OUT_PIECE
                            g0 = p * PIECE + o0
                            oeng = nc.gpsimd if h == 0 else nc.sync
                            oeng.dma_start(
                                out=yT_d[r, g0:g0 + OUT_PIECE],
                                in_=yt[:, o0:o0 + OUT_PIECE])
    nc.compile()
    return nc


def _get_nc():
    if "nc" not in _CACHE:
        _CACHE["nc"] = _build_nc()
    return _CACHE["nc"]


def _prep_inputs(x, S_raw):
    """Host-side shard + layout prep. Returns per-core input maps."""
    import ml_dtypes

    bf16 = ml_dtypes.bfloat16
    x = np.asarray(x, dtype=np.float32)
    S_raw = np.asarray(S_raw, dtype=np.float32)
    S = S_raw - S_raw.transpose(0, 2, 1)
    # (B,H,N,D) -> (H, D, B*N), token-major per head; single bf16 tensor
    xT_full = np.ascontiguousarray(x.transpose(1, 3, 0, 2)).reshape(H * D, T)
    xbf = xT_full.astype(bf16)
    eye = np.eye(D, dtype=np.float32)
    in_maps = []
    for c in range(N_CORES):
        r = c * HPC * D
        spack = np.ascontiguousarray(np.concatenate(
            [S[c * HPC + h] for h in range(HPC)] + [eye], axis=1))
        in_maps.append({
            "x": xbf[r:r + HPC * D],
            "s": spack,
        })
    return in_maps


def _postprocess(results):
    """Gather per-core yT shards back into (B, H, N, D) fp32."""
    yT_full = np.concatenate(
        [np.asarray(r["yT"], dtype=np.float32) for r in results], axis=0)
    y = yT_full.reshape(H, D, B, N).transpose(2, 0, 3, 1)
    return np.ascontiguousarray(y)


def _execute(in_maps, trace=False, **kwargs):
    _ensure_path()
    from concourse.bass_utils import run_bass_kernel_spmd

    nc = _get_nc()
    return run_bass_kernel_spmd(nc, in_maps, core_ids=list(range(N_CORES)),
                                trace=trace, **kwargs)


def kernel(x, S_raw):
    in_maps = _prep_inputs(x, S_raw)
    res = _execute(in_maps)
    return _postprocess(res.results)
